# revision 12
# baseline (speedup 1.0000x reference)
"""GraphSAGE via all-to-all neighbor-message exchange (v3).

Pipeline (3 SPMD programs, host relays the all-to-all between them):

P1  source side, layer 1: each core forms the per-edge messages x[src] for
    its OWN nodes' outgoing edges with one-hot PE matmuls from SBUF-resident
    x (no DMA descriptors per edge) and writes them contiguously, grouped by
    source tile.
H1  host all-to-all: the per-edge message rows (device-produced) are
    permuted from source order into each destination core's tile-major
    order. Pure bijective re-layout of device data; no arithmetic.
P2  destination side layer 1 + source side layer 2: contiguous reads of the
    permuted messages, S^T matmul segment-mean, dense h = ...W1..., then
    z = h@W2l^T and s = h@W2r^T + b2, and the layer-2 source-side messages
    z[src] (same one-hot machinery, z stays in SBUF).
H2  host all-to-all of the z-messages (same permutation).
P3  destination side layer 2: segment-mean of z-messages + s.

The one-hot matrices: destination side S[e, n] = (dst_slot[e] == n) is
built with edge-on-partition broadcast + iota (as in v2). Source side
needs slot-on-partition Sel[p, e] = (src_slot[e] == p): the slot sequence
is broadcast-DMA'd across partitions and compared against a partition-index
iota column.
"""
import sys
from contextlib import ExitStack

import numpy as np

for _p in ("/opt/trn_rl_repo",):
    if _p not in sys.path:
        sys.path.insert(0, _p)

import concourse.bass as bass
import concourse.tile as tile
from concourse import bacc, mybir
from concourse.bass_utils import run_bass_kernel_spmd
from concourse.masks import make_identity

try:
    import ml_dtypes
    BF16 = ml_dtypes.bfloat16
except ImportError:  # pragma: no cover
    import jax.numpy as jnp
    BF16 = jnp.bfloat16

def _ensure_axon_hooks():
    """run_bass_kernel_spmd(trace=True) imports antenv.axon_hooks, which this
    image lacks; install a ctypes-backed hook so tracing works (or degrades
    to a no-op instead of an ImportError)."""
    try:
        import antenv.axon_hooks  # noqa: F401
        return
    except ImportError:
        pass
    import contextlib
    import ctypes
    import types

    def _make_hook():
        try:
            lib = ctypes.CDLL("/opt/axon/libaxon_pjrt.so")
        except OSError:
            return None
        if not hasattr(lib, "axon_start_nrt_profile"):
            return None
        lib.axon_start_nrt_profile.argtypes = [ctypes.POINTER(ctypes.c_int64), ctypes.c_size_t]
        lib.axon_start_nrt_profile.restype = ctypes.c_int64
        lib.axon_stop_nrt_profile.argtypes = [ctypes.c_char_p]
        lib.axon_stop_nrt_profile.restype = ctypes.c_int64

        @contextlib.contextmanager
        def _hook(output_dir, device_ids):
            import jax
            jax.devices()
            if device_ids:
                ids = (ctypes.c_int64 * len(device_ids))(*device_ids)
                rc = lib.axon_start_nrt_profile(ids, len(device_ids))
            else:
                rc = lib.axon_start_nrt_profile(None, 0)
            if rc != 0:
                raise RuntimeError(f"axon_start_nrt_profile rc={rc}")
            try:
                yield
            finally:
                lib.axon_stop_nrt_profile(str(output_dir).encode())

        return _hook

    hook = _make_hook()
    mod = types.ModuleType("antenv.axon_hooks")
    mod.get_axon_ntff_profile_hook = lambda: hook
    mod.set_axon_ntff_profile_hook = lambda h: None
    import antenv
    antenv.axon_hooks = mod
    sys.modules["antenv.axon_hooks"] = mod


_ensure_axon_hooks()

N_NODES = 50000
N_EDGES = 800000
DIM_IN, DIM_H, DIM_OUT = 128, 256, 64
N_CORES = 8
P = 128
TILES_PER_CORE = 49
N_TILES = N_CORES * TILES_PER_CORE       # 392
NPAD_CORE = TILES_PER_CORE * P           # 6272
PAD_SLOT = 200.0
QCH = 8                                  # chunks per message group (DMA unit)

LAST_RESULTS = []


def _run_spmd_retry(nc, in_maps, **kw):
    import time
    try:
        return run_bass_kernel_spmd(nc, in_maps, core_ids=list(range(N_CORES)), **kw)
    except Exception:
        time.sleep(15)
        return run_bass_kernel_spmd(nc, in_maps, core_ids=list(range(N_CORES)), **kw)


def _nchp(T):
    n = TILES_PER_CORE * T
    return (n + QCH - 1) // QCH * QCH


def _partition_nodes(deg_in, deg_out):
    """Greedy 2D-balanced packing of nodes into tiles of <=128, balancing
    per-tile in-degree and out-degree sums."""
    deg = deg_in + deg_out
    order = np.argsort(-deg, kind="stable")
    sumsI = np.zeros(N_TILES, np.float64)
    sumsO = np.zeros(N_TILES, np.float64)
    counts = np.zeros(N_TILES, np.int64)
    tile_of = np.empty(N_NODES, np.int64)
    slot_of = np.empty(N_NODES, np.int64)
    capI = 2048.0   # hard in-degree target: T_in = 16
    capO = 2176.0   # soft out-degree target: T_out <= 17
    for node in order:
        dI, dO = deg_in[node], deg_out[node]
        score = np.maximum((sumsI + dI) / capI, (sumsO + dO) / capO)
        score[counts >= P] = np.inf
        t = int(np.argmin(score))
        tile_of[node] = t
        slot_of[node] = counts[t]
        counts[t] += 1
        sumsI[t] += dI
        sumsO[t] += dO
    T_in = int(np.ceil(sumsI.max() / P))
    T_out = int(np.ceil(sumsO.max() / P))
    return tile_of, slot_of, T_in, T_out


def _rank_in_groups(keys, n_groups):
    """For each element, its rank within its key-group (stable)."""
    order = np.argsort(keys, kind="stable")
    counts = np.bincount(keys, minlength=n_groups)
    starts = np.concatenate([[0], np.cumsum(counts)[:-1]])
    rank_sorted = np.arange(len(keys)) - np.repeat(starts, counts)
    rank = np.empty(len(keys), np.int64)
    rank[order] = rank_sorted
    return rank


def _build_layout(src, dst, tile_of, slot_of, T_in, T_out):
    """Edge placement metadata.

    Returns:
      gpos1[e]: global flat row of edge e in the concatenated source-order
                message array (core-major, [NCHP_OUT*128] rows per core).
      dpos2[e]: (dst_core, flat row) of edge e in the destination-order
                message array ([NCHP_IN*128] rows per core).
      srcrel:   per-core [1, NCHP_OUT*128] bf16 slot sequence (PAD on pads).
      dstrel:   per-core [P, NCHP_IN] bf16 dst-slot per edge lane.
    """
    NCHP_IN = _nchp(T_in)
    NCHP_OUT = TILES_PER_CORE * T_out
    stile, sslot = tile_of[src], slot_of[src]
    dtile, dslot = tile_of[dst], slot_of[dst]

    rank1 = _rank_in_groups(stile, N_TILES)
    fpos1 = (stile % TILES_PER_CORE) * T_out * P + rank1
    gpos1 = (stile // TILES_PER_CORE) * (NCHP_OUT * P) + fpos1

    rank2 = _rank_in_groups(dtile, N_TILES)
    fpos2 = (dtile % TILES_PER_CORE) * T_in * P + rank2
    dcore = dtile // TILES_PER_CORE

    srcrel, dstrel = [], []
    for c in range(N_CORES):
        m = (stile // TILES_PER_CORE) == c
        row = np.full(NCHP_OUT * P, PAD_SLOT, np.float32)
        row[fpos1[m]] = sslot[m]
        srcrel.append(np.ascontiguousarray(row[None, :].astype(BF16)))
        m2 = dcore == c
        flat = np.full(NCHP_IN * P, PAD_SLOT, np.float32)
        flat[fpos2[m2]] = dslot[m2]
        dstrel.append(np.ascontiguousarray(
            flat.reshape(NCHP_IN, P).T.astype(BF16)))
    return gpos1, (dcore, fpos2), srcrel, dstrel


# ---------------------------------------------------------------- programs

def _source_side(nc, tc, ctx, srcrel_d, rhs_fn, m_out, T_out,
                 fdim, iota_col, prefix="", ps_bufs=2, copy_split=False):
    """Emit the source-side message formation, transposed: per source tile,
    broadcast-load the slot sequence, build Sel[p, e] = (p == slot[e])
    (split across vector and gpsimd engines), then msgsT = X_s^T @ Sel in
    two wide matmuls, cast each half to bf16 and DMA columns of
    m_out [fdim, 49*T_out*128]."""
    dt = mybir.dt
    NT = T_out * P
    BLK = 4 * P                           # matmul block: one PSUM bank of fp32
    repp = ctx.enter_context(tc.tile_pool(name=f"{prefix}repp", bufs=3))
    selp = ctx.enter_context(tc.tile_pool(name=f"{prefix}selp", bufs=3))
    mb = ctx.enter_context(tc.tile_pool(name=f"{prefix}mb", bufs=4))
    psM = ctx.enter_context(tc.tile_pool(name=f"{prefix}psM", bufs=ps_bufs,
                                         space="PSUM"))
    k = 0
    for s in range(TILES_PER_CORE):
        rep = repp.tile([P, NT], dt.bfloat16, name=f"{prefix}rep")
        nc.sync.dma_start(
            rep[:], srcrel_d[0:1, s * NT:(s + 1) * NT].to_broadcast([P, NT]))
        sel = selp.tile([P, NT], dt.bfloat16, name=f"{prefix}sel")
        nc.vector.tensor_tensor(
            out=sel[:], in0=iota_col.to_broadcast([P, NT]), in1=rep[:],
            op=mybir.AluOpType.is_equal)
        for c0 in range(0, NT, BLK):
            c1 = min(c0 + BLK, NT)
            ncols = c1 - c0
            ps = psM.tile([fdim, BLK], dt.float32, name=f"{prefix}mps")
            nc.tensor.matmul(
                out=ps[:, :ncols], lhsT=rhs_fn(s), rhs=sel[:, c0:c1],
                start=True, stop=True)
            mt = mb.tile([fdim, BLK], dt.bfloat16, name=f"{prefix}mt")
            if copy_split and k % 2 == 0:
                nc.vector.tensor_copy(mt[:, :ncols], ps[:, :ncols])
            else:
                nc.scalar.mul(mt[:, :ncols], ps[:, :ncols], 1.0)
            nc.sync.dma_start(m_out[:, s * NT + c0:s * NT + c1],
                              mt[:, :ncols])
            k += 1


def _dest_stream(nc, tc, ctx, m_in, dst_sb, iota_big, T_in, fdim, prefix=""):
    """Streaming loader for the destination side: groups of QCH chunks are
    DMA'd from the permuted message array and their S blocks built.
    Returns ensure(chunk) -> (msgs_tile, S_tile, slot)."""
    dt = mybir.dt
    mp = ctx.enter_context(tc.tile_pool(name=f"{prefix}mp", bufs=4))
    sp = ctx.enter_context(tc.tile_pool(name=f"{prefix}sp", bufs=4))
    state = {"next": 0, "msgs": {}, "S": {}}

    def issue(g):
        msgs = mp.tile([P, QCH, fdim], dt.bfloat16, name=f"{prefix}msgs")
        nc.sync.dma_start(msgs[:], m_in[:, g * QCH:(g + 1) * QCH, :])
        S = sp.tile([P, QCH * P], dt.bfloat16, name=f"{prefix}S")
        try:
            nc.vector.tensor_tensor(
                out=S[:],
                in0=dst_sb[:, g * QCH:(g + 1) * QCH, None].to_broadcast(
                    [P, QCH, P]),
                in1=iota_big[:],
                op=mybir.AluOpType.is_equal)
        except Exception:
            for j in range(QCH):
                nc.vector.tensor_tensor(
                    out=S[:, j * P:(j + 1) * P],
                    in0=dst_sb[:, g * QCH + j:g * QCH + j + 1].to_broadcast([P, P]),
                    in1=iota_big[:, :P],
                    op=mybir.AluOpType.is_equal)
        state["msgs"][g] = msgs
        state["S"][g] = S

    def ensure(chunk):
        g = chunk // QCH
        while state["next"] <= g:
            issue(state["next"])
            state["next"] += 1
        return state["msgs"][g], state["S"][g], chunk % QCH

    return ensure


def _load_basics(nc, const, T_in):
    """iota tiles, dstrel, deg/recip. Returns dict."""
    dt = mybir.dt
    NCHP_IN = _nchp(T_in)
    dstrel = nc.dram_tensor("dstrel", [P, NCHP_IN], dt.bfloat16,
                            kind="ExternalInput").ap()
    deg_col = nc.dram_tensor("deg_col", [P, TILES_PER_CORE], dt.float32,
                             kind="ExternalInput").ap()
    s = {}
    dst_sb = const.tile([P, NCHP_IN], dt.bfloat16)
    nc.sync.dma_start(dst_sb[:], dstrel[:, :])
    s["dst"] = dst_sb
    deg_sb = const.tile([P, TILES_PER_CORE], dt.float32)
    nc.sync.dma_start(deg_sb[:], deg_col[:, :])
    iota_f = const.tile([P, P], dt.float32)
    nc.gpsimd.iota(iota_f[:], pattern=[[1, P]], base=0, channel_multiplier=0,
                   allow_small_or_imprecise_dtypes=True)
    iota_sm = const.tile([P, P], dt.bfloat16)
    nc.vector.tensor_copy(iota_sm[:], iota_f[:])
    iota_big = const.tile([P, QCH * P], dt.bfloat16)
    for _j in range(QCH):
        nc.vector.tensor_copy(iota_big[:, _j * P:(_j + 1) * P], iota_sm[:])
    s["iota_big"] = iota_big
    iotac_f = const.tile([P, 1], dt.float32)
    nc.gpsimd.iota(iotac_f[:], pattern=[[0, 1]], base=0, channel_multiplier=1,
                   allow_small_or_imprecise_dtypes=True)
    iota_col = const.tile([P, 1], dt.bfloat16)
    nc.vector.tensor_copy(iota_col[:], iotac_f[:])
    s["iota_col"] = iota_col
    recip = const.tile([P, TILES_PER_CORE], dt.float32)
    nc.vector.tensor_scalar_max(recip[:], deg_sb[:], 1.0)
    nc.vector.reciprocal(recip[:], recip[:])
    s["recip"] = recip
    return s


def _build_p1(T_out):
    """P1: layer-1 source-side messages."""
    dt = mybir.dt
    NCHP_OUT = TILES_PER_CORE * T_out
    nc = bacc.Bacc("TRN2", target_bir_lowering=False, debug=False,
                   enable_asserts=False, num_devices=N_CORES)
    selfX = nc.dram_tensor("selfX", [P, NPAD_CORE], dt.bfloat16,
                           kind="ExternalInput").ap()
    srcrel = nc.dram_tensor("srcrel", [1, NCHP_OUT * P], dt.bfloat16,
                            kind="ExternalInput").ap()
    m_out = nc.dram_tensor("m_out", [DIM_IN, NCHP_OUT * P], dt.bfloat16,
                           kind="ExternalOutput").ap()
    with tile.TileContext(nc) as tc:
        with ExitStack() as ctx:
            const = ctx.enter_context(tc.tile_pool(name="const", bufs=1))
            selfX_sb = const.tile([P, NPAD_CORE], dt.bfloat16)
            nc.sync.dma_start(selfX_sb[:], selfX[:, :])
            iotac_f = const.tile([P, 1], dt.float32)
            nc.gpsimd.iota(iotac_f[:], pattern=[[0, 1]], base=0,
                           channel_multiplier=1,
                           allow_small_or_imprecise_dtypes=True)
            iota_col = const.tile([P, 1], dt.bfloat16)
            nc.vector.tensor_copy(iota_col[:], iotac_f[:])
            _source_side(nc, tc, ctx, srcrel,
                         lambda s: selfX_sb[:, s * P:(s + 1) * P],
                         m_out, T_out, DIM_IN, iota_col[:], copy_split=True, ps_bufs=4)
    nc.compile()
    return nc


def _build_p2(T_in, T_out):
    """P2: layer-1 destination side + dense + layer-2 pre-transforms +
    layer-2 source-side messages."""
    dt = mybir.dt
    NCHP_IN = _nchp(T_in)
    NCHP_OUT = TILES_PER_CORE * T_out
    nc = bacc.Bacc("TRN2", target_bir_lowering=False, debug=False,
                   enable_asserts=False, num_devices=N_CORES)
    m_in = nc.dram_tensor("m_in", [P, NCHP_IN, DIM_IN], dt.bfloat16,
                          kind="ExternalInput").ap()
    selfT = nc.dram_tensor("selfT", [P, NPAD_CORE], dt.bfloat16,
                           kind="ExternalInput").ap()
    srcrel = nc.dram_tensor("srcrel", [1, NCHP_OUT * P], dt.bfloat16,
                            kind="ExternalInput").ap()
    w1l = nc.dram_tensor("w1l", [P, DIM_H], dt.bfloat16, kind="ExternalInput").ap()
    w1r = nc.dram_tensor("w1r", [P, DIM_H], dt.bfloat16, kind="ExternalInput").ap()
    b1 = nc.dram_tensor("b1", [P, 2], dt.float32, kind="ExternalInput").ap()
    w2l = nc.dram_tensor("w2l", [P, 2 * DIM_OUT], dt.bfloat16, kind="ExternalInput").ap()
    w2r = nc.dram_tensor("w2r", [P, 2 * DIM_OUT], dt.bfloat16, kind="ExternalInput").ap()
    b2 = nc.dram_tensor("b2", [1, DIM_OUT], dt.bfloat16, kind="ExternalInput").ap()
    mz_out = nc.dram_tensor("mz_out", [DIM_OUT, NCHP_OUT * P], dt.bfloat16,
                            kind="ExternalOutput").ap()
    s_out = nc.dram_tensor("s_out", [NPAD_CORE, DIM_OUT], dt.float32,
                           kind="ExternalOutput").ap()
    with tile.TileContext(nc) as tc:
        with ExitStack() as ctx:
            const = ctx.enter_context(tc.tile_pool(name="const", bufs=1))
            work = ctx.enter_context(tc.tile_pool(name="work", bufs=3))
            outp = ctx.enter_context(tc.tile_pool(name="outp", bufs=4))
            psA = ctx.enter_context(tc.tile_pool(name="psA", bufs=2, space="PSUM"))
            psT = ctx.enter_context(tc.tile_pool(name="psT", bufs=1, space="PSUM"))
            psH = ctx.enter_context(tc.tile_pool(name="psH", bufs=1, space="PSUM"))
            psZ = ctx.enter_context(tc.tile_pool(name="psZ", bufs=1, space="PSUM"))

            sb = _load_basics(nc, const, T_in)
            w1l_sb = const.tile([P, DIM_H], dt.bfloat16)
            nc.sync.dma_start(w1l_sb[:], w1l[:, :])
            w1r_sb = const.tile([P, DIM_H], dt.bfloat16)
            nc.sync.dma_start(w1r_sb[:], w1r[:, :])
            b1_sb = const.tile([P, 2], dt.float32)
            nc.sync.dma_start(b1_sb[:], b1[:, :])
            w2l_sb = const.tile([P, 2 * DIM_OUT], dt.bfloat16)
            nc.sync.dma_start(w2l_sb[:], w2l[:, :])
            w2r_sb = const.tile([P, 2 * DIM_OUT], dt.bfloat16)
            nc.sync.dma_start(w2r_sb[:], w2r[:, :])
            b2_sb = const.tile([1, DIM_OUT], dt.bfloat16)
            nc.sync.dma_start(b2_sb[:], b2[:, :])
            self_sb = const.tile([P, NPAD_CORE], dt.bfloat16)
            nc.sync.dma_start(self_sb[:], selfT[:, :])
            ident = const.tile([P, P], dt.bfloat16)
            make_identity(nc, ident[:])
            ones1 = const.tile([1, P], dt.bfloat16)
            nc.gpsimd.memset(ones1[:], 1.0)
            zall = const.tile([P, TILES_PER_CORE, DIM_OUT], dt.bfloat16)

            ensure = _dest_stream(nc, tc, ctx, m_in, sb["dst"], sb["iota_big"][:],
                                  T_in, DIM_IN)

            for t in range(TILES_PER_CORE):
                agg_ps = psA.tile([P, DIM_IN], dt.float32)
                for j in range(T_in):
                    msgs, S, slot = ensure(t * T_in + j)
                    nc.tensor.matmul(
                        out=agg_ps[:],
                        lhsT=S[:, slot * P:(slot + 1) * P],
                        rhs=msgs[:, slot, :],
                        start=(j == 0), stop=(j == T_in - 1))
                agg_sb = work.tile([P, DIM_IN], dt.bfloat16)
                nc.scalar.mul(agg_sb[:], agg_ps[:], sb["recip"][:, t:t + 1])
                tp = psT.tile([P, P], dt.bfloat16)
                nc.tensor.transpose(out=tp[:], in_=agg_sb[:], identity=ident[:])
                aggT_sb = work.tile([P, P], dt.bfloat16)
                nc.vector.tensor_copy(aggT_sb[:], tp[:])
                hT_sb = work.tile([P, 2, P], dt.bfloat16)
                for so in range(2):
                    h_ps = psH.tile([P, P], dt.float32)
                    nc.tensor.matmul(
                        out=h_ps[:], lhsT=w1l_sb[:, so * P:(so + 1) * P],
                        rhs=aggT_sb[:], start=True, stop=False)
                    nc.tensor.matmul(
                        out=h_ps[:], lhsT=w1r_sb[:, so * P:(so + 1) * P],
                        rhs=self_sb[:, t * P:(t + 1) * P],
                        start=False, stop=True)
                    nc.scalar.activation(
                        hT_sb[:, so, :], h_ps[:],
                        mybir.ActivationFunctionType.Relu,
                        bias=b1_sb[:, so:so + 1], scale=1.0)
                z_ps = psZ.tile([P, DIM_OUT], dt.float32, name="zs_ps")
                for si in range(2):
                    nc.tensor.matmul(
                        out=z_ps[:], lhsT=hT_sb[:, si, :],
                        rhs=w2l_sb[:, si * DIM_OUT:(si + 1) * DIM_OUT],
                        start=(si == 0), stop=(si == 1))
                nc.scalar.mul(zall[:, t, :], z_ps[:], 1.0)
                s_ps = psZ.tile([P, DIM_OUT], dt.float32, name="zs_ps")
                for si in range(2):
                    nc.tensor.matmul(
                        out=s_ps[:], lhsT=hT_sb[:, si, :],
                        rhs=w2r_sb[:, si * DIM_OUT:(si + 1) * DIM_OUT],
                        start=(si == 0), stop=False)
                nc.tensor.matmul(
                    out=s_ps[:], lhsT=ones1[:], rhs=b2_sb[:],
                    start=False, stop=True)
                s_sb = outp.tile([P, DIM_OUT], dt.float32)
                nc.scalar.mul(s_sb[:], s_ps[:], 1.0)
                nc.sync.dma_start(s_out[t * P:(t + 1) * P, :], s_sb[:])

            # layer-2 source side: z messages from SBUF-resident zall
            _source_side(nc, tc, ctx, srcrel,
                         lambda s: zall[:, s, :],
                         mz_out, T_out, DIM_OUT, sb["iota_col"][:],
                         prefix="z", ps_bufs=2, copy_split=True)
    nc.compile()
    return nc


def _build_p3(T_in):
    """P3: layer-2 destination side."""
    dt = mybir.dt
    NCHP_IN = _nchp(T_in)
    nc = bacc.Bacc("TRN2", target_bir_lowering=False, debug=False,
                   enable_asserts=False, num_devices=N_CORES)
    m_in = nc.dram_tensor("m_in", [P, NCHP_IN, DIM_OUT], dt.bfloat16,
                          kind="ExternalInput").ap()
    s_in = nc.dram_tensor("s_in", [NPAD_CORE, DIM_OUT], dt.float32,
                          kind="ExternalInput").ap()
    out = nc.dram_tensor("out", [NPAD_CORE, DIM_OUT], dt.float32,
                         kind="ExternalOutput").ap()
    with tile.TileContext(nc) as tc:
        with ExitStack() as ctx:
            const = ctx.enter_context(tc.tile_pool(name="const", bufs=1))
            sload = ctx.enter_context(tc.tile_pool(name="sload", bufs=3))
            outp = ctx.enter_context(tc.tile_pool(name="outp", bufs=4))
            psA = ctx.enter_context(tc.tile_pool(name="psA", bufs=2, space="PSUM"))
            sb = _load_basics(nc, const, T_in)
            ensure = _dest_stream(nc, tc, ctx, m_in, sb["dst"], sb["iota_big"][:],
                                  T_in, DIM_OUT)
            for t in range(TILES_PER_CORE):
                agg_ps = psA.tile([P, DIM_OUT], dt.float32)
                for j in range(T_in):
                    msgs, S, slot = ensure(t * T_in + j)
                    nc.tensor.matmul(
                        out=agg_ps[:],
                        lhsT=S[:, slot * P:(slot + 1) * P],
                        rhs=msgs[:, slot, :],
                        start=(j == 0), stop=(j == T_in - 1))
                s_tile = sload.tile([P, DIM_OUT], dt.float32)
                nc.sync.dma_start(s_tile[:], s_in[t * P:(t + 1) * P, :])
                agg_sb = outp.tile([P, DIM_OUT], dt.float32)
                nc.scalar.mul(agg_sb[:], agg_ps[:], sb["recip"][:, t:t + 1])
                o_sb = outp.tile([P, DIM_OUT], dt.float32)
                nc.vector.tensor_add(o_sb[:], agg_sb[:], s_tile[:])
                nc.sync.dma_start(out[t * P:(t + 1) * P, :], o_sb[:])
    nc.compile()
    return nc


_PROG_CACHE = {}


def _get_programs(T_in, T_out):
    key = (T_in, T_out)
    if key not in _PROG_CACHE:
        _PROG_CACHE[key] = (_build_p1(T_out), _build_p2(T_in, T_out),
                            _build_p3(T_in))
    return _PROG_CACHE[key]


def _permute(m_results, gpos1, dpos2, T_in, fdim):
    """Host all-to-all: source-order message rows -> per-destination-core
    tile-major arrays [128, NCHP_IN, fdim] (transposed-chunk layout)."""
    NCHP_IN = _nchp(T_in)
    flat = np.concatenate(
        [np.ascontiguousarray(m.T) for m in m_results], axis=0)
    dcore, fpos2 = dpos2
    outs = []
    for d in range(N_CORES):
        m = dcore == d
        arr = np.zeros((NCHP_IN * P, fdim), flat.dtype)
        arr[fpos2[m]] = flat[gpos1[m]]
        outs.append(np.ascontiguousarray(
            arr.reshape(NCHP_IN, P, fdim).transpose(1, 0, 2)))
    return outs


def kernel(x, edge_index, W1l, W1r, b1, W2l, W2r, b2):
    global LAST_RESULTS
    LAST_RESULTS = []
    x = np.asarray(x, np.float32)
    src = np.asarray(edge_index[0], np.int64)
    dst = np.asarray(edge_index[1], np.int64)

    deg_in = np.bincount(dst, minlength=N_NODES)
    deg_out = np.bincount(src, minlength=N_NODES)
    tile_of, slot_of, T_in, T_out = _partition_nodes(deg_in, deg_out)
    gpos1, dpos2, srcrel, dstrel = _build_layout(
        src, dst, tile_of, slot_of, T_in, T_out)

    p1, p2, p3 = _get_programs(T_in, T_out)

    trace = bool(int(__import__("os").environ.get("BASS_TRACE", "0") or 0))
    tkw = dict(trace=True, tmpdir=None) if trace else {}

    x_bf = x.astype(BF16)
    deg_cols, selfTs, selfXs, node_lists, local_lists = [], [], [], [], []
    for c in range(N_CORES):
        tiles = np.arange(c * TILES_PER_CORE, (c + 1) * TILES_PER_CORE)
        mask = np.isin(tile_of, tiles)
        nodes = np.nonzero(mask)[0]
        local_tile = tile_of[nodes] - c * TILES_PER_CORE
        local = local_tile * P + slot_of[nodes]
        dcol = np.zeros((P, TILES_PER_CORE), np.float32)
        dcol[slot_of[nodes], local_tile] = deg_in[nodes]
        sN = np.zeros((NPAD_CORE, DIM_IN), BF16)
        sN[local] = x_bf[nodes]
        deg_cols.append(dcol)
        selfTs.append(np.ascontiguousarray(sN.T))
        selfXs.append(np.ascontiguousarray(
            sN.reshape(TILES_PER_CORE, P, DIM_IN).transpose(1, 0, 2)
            .reshape(P, NPAD_CORE)))
        node_lists.append(nodes)
        local_lists.append(local)

    w1l_p = np.ascontiguousarray(np.asarray(W1l, np.float32).T).astype(BF16)
    w1r_p = np.ascontiguousarray(np.asarray(W1r, np.float32).T).astype(BF16)
    b1_np = np.asarray(b1, np.float32)
    b1_p = np.stack([b1_np[:P], b1_np[P:]], axis=1).astype(np.float32)
    w2l_np = np.asarray(W2l, np.float32)
    w2r_np = np.asarray(W2r, np.float32)
    w2l_p = np.ascontiguousarray(np.hstack([w2l_np.T[:P, :], w2l_np.T[P:, :]])).astype(BF16)
    w2r_p = np.ascontiguousarray(np.hstack([w2r_np.T[:P, :], w2r_np.T[P:, :]])).astype(BF16)
    b2_p = np.asarray(b2, np.float32)[None, :].astype(BF16)

    r1 = _run_spmd_retry(p1, [
        {"selfX": selfXs[c], "srcrel": srcrel[c]} for c in range(N_CORES)], **tkw)
    LAST_RESULTS.append(r1)

    m1 = _permute([r1.results[c]["m_out"] for c in range(N_CORES)],
                  gpos1, dpos2, T_in, DIM_IN)

    r2 = _run_spmd_retry(p2, [
        {"m_in": m1[c], "selfT": selfTs[c], "srcrel": srcrel[c],
         "dstrel": dstrel[c], "deg_col": deg_cols[c],
         "w1l": w1l_p, "w1r": w1r_p, "b1": b1_p,
         "w2l": w2l_p, "w2r": w2r_p, "b2": b2_p} for c in range(N_CORES)], **tkw)
    LAST_RESULTS.append(r2)

    m2 = _permute([r2.results[c]["mz_out"] for c in range(N_CORES)],
                  gpos1, dpos2, T_in, DIM_OUT)

    r3 = _run_spmd_retry(p3, [
        {"m_in": m2[c], "s_in": r2.results[c]["s_out"],
         "dstrel": dstrel[c], "deg_col": deg_cols[c]} for c in range(N_CORES)], **tkw)
    LAST_RESULTS.append(r3)

    out = np.zeros((N_NODES, DIM_OUT), np.float32)
    for c in range(N_CORES):
        out[node_lists[c]] = r3.results[c]["out"][local_lists[c]]
    return out


# revision 13
# speedup vs baseline: 1.1812x; 1.1812x over previous
"""GraphSAGE via all-to-all neighbor-message exchange (v3).

Pipeline (3 SPMD programs, host relays the all-to-all between them):

P1  source side, layer 1: each core forms the per-edge messages x[src] for
    its OWN nodes' outgoing edges with one-hot PE matmuls from SBUF-resident
    x (no DMA descriptors per edge) and writes them contiguously, grouped by
    source tile.
H1  host all-to-all: the per-edge message rows (device-produced) are
    permuted from source order into each destination core's tile-major
    order. Pure bijective re-layout of device data; no arithmetic.
P2  destination side layer 1 + source side layer 2: contiguous reads of the
    permuted messages, S^T matmul segment-mean, dense h = ...W1..., then
    z = h@W2l^T and s = h@W2r^T + b2, and the layer-2 source-side messages
    z[src] (same one-hot machinery, z stays in SBUF).
H2  host all-to-all of the z-messages (same permutation).
P3  destination side layer 2: segment-mean of z-messages + s.

The one-hot matrices: destination side S[e, n] = (dst_slot[e] == n) is
built with edge-on-partition broadcast + iota (as in v2). Source side
needs slot-on-partition Sel[p, e] = (src_slot[e] == p): the slot sequence
is broadcast-DMA'd across partitions and compared against a partition-index
iota column.
"""
import sys
from contextlib import ExitStack

import numpy as np

for _p in ("/opt/trn_rl_repo",):
    if _p not in sys.path:
        sys.path.insert(0, _p)

import concourse.bass as bass
import concourse.tile as tile
from concourse import bacc, mybir
from concourse.bass_utils import run_bass_kernel_spmd
from concourse.masks import make_identity

try:
    import ml_dtypes
    BF16 = ml_dtypes.bfloat16
except ImportError:  # pragma: no cover
    import jax.numpy as jnp
    BF16 = jnp.bfloat16

def _ensure_axon_hooks():
    """run_bass_kernel_spmd(trace=True) imports antenv.axon_hooks, which this
    image lacks; install a ctypes-backed hook so tracing works (or degrades
    to a no-op instead of an ImportError)."""
    try:
        import antenv.axon_hooks  # noqa: F401
        return
    except ImportError:
        pass
    import contextlib
    import ctypes
    import types

    def _make_hook():
        try:
            lib = ctypes.CDLL("/opt/axon/libaxon_pjrt.so")
        except OSError:
            return None
        if not hasattr(lib, "axon_start_nrt_profile"):
            return None
        lib.axon_start_nrt_profile.argtypes = [ctypes.POINTER(ctypes.c_int64), ctypes.c_size_t]
        lib.axon_start_nrt_profile.restype = ctypes.c_int64
        lib.axon_stop_nrt_profile.argtypes = [ctypes.c_char_p]
        lib.axon_stop_nrt_profile.restype = ctypes.c_int64

        @contextlib.contextmanager
        def _hook(output_dir, device_ids):
            import jax
            jax.devices()
            if device_ids:
                ids = (ctypes.c_int64 * len(device_ids))(*device_ids)
                rc = lib.axon_start_nrt_profile(ids, len(device_ids))
            else:
                rc = lib.axon_start_nrt_profile(None, 0)
            if rc != 0:
                raise RuntimeError(f"axon_start_nrt_profile rc={rc}")
            try:
                yield
            finally:
                lib.axon_stop_nrt_profile(str(output_dir).encode())

        return _hook

    hook = _make_hook()
    mod = types.ModuleType("antenv.axon_hooks")
    mod.get_axon_ntff_profile_hook = lambda: hook
    mod.set_axon_ntff_profile_hook = lambda h: None
    import antenv
    antenv.axon_hooks = mod
    sys.modules["antenv.axon_hooks"] = mod


_ensure_axon_hooks()

N_NODES = 50000
N_EDGES = 800000
DIM_IN, DIM_H, DIM_OUT = 128, 256, 64
N_CORES = 8
P = 128
TILES_PER_CORE = 49
N_TILES = N_CORES * TILES_PER_CORE       # 392
NPAD_CORE = TILES_PER_CORE * P           # 6272
PAD_SLOT = 200.0
QCH = 8                                  # chunks per message group (DMA unit)

LAST_RESULTS = []


def _run_spmd_retry(nc, in_maps, **kw):
    import time
    try:
        return run_bass_kernel_spmd(nc, in_maps, core_ids=list(range(N_CORES)), **kw)
    except Exception:
        time.sleep(15)
        return run_bass_kernel_spmd(nc, in_maps, core_ids=list(range(N_CORES)), **kw)


def _nchp(T):
    n = TILES_PER_CORE * T
    return (n + QCH - 1) // QCH * QCH


def _partition_nodes(deg_in, deg_out):
    """Greedy 2D-balanced packing of nodes into tiles of <=128, balancing
    per-tile in-degree and out-degree sums."""
    deg = deg_in + deg_out
    order = np.argsort(-deg, kind="stable")
    sumsI = np.zeros(N_TILES, np.float64)
    sumsO = np.zeros(N_TILES, np.float64)
    counts = np.zeros(N_TILES, np.int64)
    tile_of = np.empty(N_NODES, np.int64)
    slot_of = np.empty(N_NODES, np.int64)
    capI = float(deg_in.sum()) / N_TILES
    capO = float(deg_out.sum()) / N_TILES
    for node in order:
        dI, dO = deg_in[node], deg_out[node]
        score = np.maximum((sumsI + dI) / capI, (sumsO + dO) / capO)
        score[counts >= P] = np.inf
        t = int(np.argmin(score))
        tile_of[node] = t
        slot_of[node] = counts[t]
        counts[t] += 1
        sumsI[t] += dI
        sumsO[t] += dO
    T_in = int(np.ceil(sumsI.max() / P))
    T_out = int(np.ceil(sumsO.max() / P))
    return tile_of, slot_of, T_in, T_out


def _rank_in_groups(keys, n_groups):
    """For each element, its rank within its key-group (stable)."""
    order = np.argsort(keys, kind="stable")
    counts = np.bincount(keys, minlength=n_groups)
    starts = np.concatenate([[0], np.cumsum(counts)[:-1]])
    rank_sorted = np.arange(len(keys)) - np.repeat(starts, counts)
    rank = np.empty(len(keys), np.int64)
    rank[order] = rank_sorted
    return rank


def _build_layout(src, dst, tile_of, slot_of, T_in, T_out):
    """Edge placement metadata.

    Returns:
      gpos1[e]: global flat row of edge e in the concatenated source-order
                message array (core-major, [NCHP_OUT*128] rows per core).
      dpos2[e]: (dst_core, flat row) of edge e in the destination-order
                message array ([NCHP_IN*128] rows per core).
      srcrel:   per-core [1, NCHP_OUT*128] bf16 slot sequence (PAD on pads).
      dstrel:   per-core [P, NCHP_IN] bf16 dst-slot per edge lane.
    """
    NCHP_IN, NCHP_OUT = _nchp(T_in), _nchp(T_out)
    stile, sslot = tile_of[src], slot_of[src]
    dtile, dslot = tile_of[dst], slot_of[dst]

    rank1 = _rank_in_groups(stile, N_TILES)
    fpos1 = (stile % TILES_PER_CORE) * T_out * P + rank1
    gpos1 = (stile // TILES_PER_CORE) * (NCHP_OUT * P) + fpos1

    rank2 = _rank_in_groups(dtile, N_TILES)
    fpos2 = (dtile % TILES_PER_CORE) * T_in * P + rank2
    dcore = dtile // TILES_PER_CORE

    srcrel, dstrel = [], []
    for c in range(N_CORES):
        m = (stile // TILES_PER_CORE) == c
        row = np.full(NCHP_OUT * P, PAD_SLOT, np.float32)
        row[fpos1[m]] = sslot[m]
        srcrel.append(np.ascontiguousarray(row[None, :].astype(BF16)))
        m2 = dcore == c
        flat = np.full(NCHP_IN * P, PAD_SLOT, np.float32)
        flat[fpos2[m2]] = dslot[m2]
        dstrel.append(np.ascontiguousarray(
            flat.reshape(NCHP_IN, P).T.astype(BF16)))
    return gpos1, (dcore, fpos2), srcrel, dstrel


# ---------------------------------------------------------------- programs

def _source_side(nc, tc, ctx, srcrel_d, rhs_fn, m_out, T_out,
                 fdim, iota_col, prefix=""):
    """Emit the source-side message formation: for each QCH-chunk group,
    broadcast-load the slot sequence, build Sel[p, e] = (p == slot[e]),
    matmul per chunk against the source tile's rows, copy the group's PSUM
    to bf16 SBUF (alternating scalar/vector), and DMA out."""
    dt = mybir.dt
    NCHP_OUT = _nchp(T_out)
    repp = ctx.enter_context(tc.tile_pool(name=f"{prefix}repp", bufs=3))
    selp = ctx.enter_context(tc.tile_pool(name=f"{prefix}selp", bufs=3))
    mb = ctx.enter_context(tc.tile_pool(name=f"{prefix}mb", bufs=3))
    psM = ctx.enter_context(tc.tile_pool(name=f"{prefix}psM", bufs=2, space="PSUM"))
    for g in range(NCHP_OUT // QCH):
        rep = repp.tile([P, QCH * P], dt.bfloat16, name=f"{prefix}rep")
        nc.sync.dma_start(
            rep[:], srcrel_d[0:1, g * QCH * P:(g + 1) * QCH * P]
            .to_broadcast([P, QCH * P]))
        sel = selp.tile([P, QCH * P], dt.bfloat16, name=f"{prefix}sel")
        nc.vector.tensor_tensor(
            out=sel[:], in0=iota_col.to_broadcast([P, QCH * P]), in1=rep[:],
            op=mybir.AluOpType.is_equal)
        ps = psM.tile([P, QCH, fdim], dt.float32, name=f"{prefix}mps")
        for jj in range(QCH):
            j = g * QCH + jj
            s = min(j // T_out, TILES_PER_CORE - 1)
            nc.tensor.matmul(
                out=ps[:, jj, :],
                lhsT=sel[:, jj * P:(jj + 1) * P],
                rhs=rhs_fn(s),
                start=True, stop=True)
        mt = mb.tile([P, QCH, fdim], dt.bfloat16, name=f"{prefix}mt")
        if g % 2 == 0:
            nc.vector.tensor_copy(mt[:], ps[:])
        else:
            nc.scalar.mul(mt[:], ps[:], 1.0)
        nc.sync.dma_start(m_out[:, g * QCH:(g + 1) * QCH, :], mt[:])


def _dest_stream(nc, tc, ctx, m_in, dst_sb, iota_big, T_in, fdim, prefix=""):
    """Streaming loader for the destination side: groups of QCH chunks are
    DMA'd from the permuted message array and their S blocks built.
    Returns ensure(chunk) -> (msgs_tile, S_tile, slot)."""
    dt = mybir.dt
    mp = ctx.enter_context(tc.tile_pool(name=f"{prefix}mp", bufs=4))
    sp = ctx.enter_context(tc.tile_pool(name=f"{prefix}sp", bufs=4))
    state = {"next": 0, "msgs": {}, "S": {}}

    def issue(g):
        msgs = mp.tile([P, QCH, fdim], dt.bfloat16, name=f"{prefix}msgs")
        nc.sync.dma_start(msgs[:], m_in[:, g * QCH:(g + 1) * QCH, :])
        S = sp.tile([P, QCH * P], dt.bfloat16, name=f"{prefix}S")
        try:
            nc.vector.tensor_tensor(
                out=S[:],
                in0=dst_sb[:, g * QCH:(g + 1) * QCH, None].to_broadcast(
                    [P, QCH, P]),
                in1=iota_big[:],
                op=mybir.AluOpType.is_equal)
        except Exception:
            for j in range(QCH):
                nc.vector.tensor_tensor(
                    out=S[:, j * P:(j + 1) * P],
                    in0=dst_sb[:, g * QCH + j:g * QCH + j + 1].to_broadcast([P, P]),
                    in1=iota_big[:, :P],
                    op=mybir.AluOpType.is_equal)
        state["msgs"][g] = msgs
        state["S"][g] = S

    def ensure(chunk):
        g = chunk // QCH
        while state["next"] <= g:
            issue(state["next"])
            state["next"] += 1
        return state["msgs"][g], state["S"][g], chunk % QCH

    return ensure


def _load_basics(nc, const, T_in):
    """iota tiles, dstrel, deg/recip. Returns dict."""
    dt = mybir.dt
    NCHP_IN = _nchp(T_in)
    dstrel = nc.dram_tensor("dstrel", [P, NCHP_IN], dt.bfloat16,
                            kind="ExternalInput").ap()
    deg_col = nc.dram_tensor("deg_col", [P, TILES_PER_CORE], dt.float32,
                             kind="ExternalInput").ap()
    s = {}
    dst_sb = const.tile([P, NCHP_IN], dt.bfloat16)
    nc.sync.dma_start(dst_sb[:], dstrel[:, :])
    s["dst"] = dst_sb
    deg_sb = const.tile([P, TILES_PER_CORE], dt.float32)
    nc.sync.dma_start(deg_sb[:], deg_col[:, :])
    iota_f = const.tile([P, P], dt.float32)
    nc.gpsimd.iota(iota_f[:], pattern=[[1, P]], base=0, channel_multiplier=0,
                   allow_small_or_imprecise_dtypes=True)
    iota_sm = const.tile([P, P], dt.bfloat16)
    nc.vector.tensor_copy(iota_sm[:], iota_f[:])
    iota_big = const.tile([P, QCH * P], dt.bfloat16)
    for _j in range(QCH):
        nc.vector.tensor_copy(iota_big[:, _j * P:(_j + 1) * P], iota_sm[:])
    s["iota_big"] = iota_big
    iotac_f = const.tile([P, 1], dt.float32)
    nc.gpsimd.iota(iotac_f[:], pattern=[[0, 1]], base=0, channel_multiplier=1,
                   allow_small_or_imprecise_dtypes=True)
    iota_col = const.tile([P, 1], dt.bfloat16)
    nc.vector.tensor_copy(iota_col[:], iotac_f[:])
    s["iota_col"] = iota_col
    recip = const.tile([P, TILES_PER_CORE], dt.float32)
    nc.vector.tensor_scalar_max(recip[:], deg_sb[:], 1.0)
    nc.vector.reciprocal(recip[:], recip[:])
    s["recip"] = recip
    return s


def _build_p1(T_out):
    """P1: layer-1 source-side messages."""
    dt = mybir.dt
    NCHP_OUT = _nchp(T_out)
    nc = bacc.Bacc("TRN2", target_bir_lowering=False, debug=False,
                   enable_asserts=False, num_devices=N_CORES)
    selfX = nc.dram_tensor("selfX", [P, NPAD_CORE], dt.bfloat16,
                           kind="ExternalInput").ap()
    srcrel = nc.dram_tensor("srcrel", [1, NCHP_OUT * P], dt.bfloat16,
                            kind="ExternalInput").ap()
    m_out = nc.dram_tensor("m_out", [P, NCHP_OUT, DIM_IN], dt.bfloat16,
                           kind="ExternalOutput").ap()
    with tile.TileContext(nc) as tc:
        with ExitStack() as ctx:
            const = ctx.enter_context(tc.tile_pool(name="const", bufs=1))
            selfX_sb = const.tile([P, NPAD_CORE], dt.bfloat16)
            nc.sync.dma_start(selfX_sb[:], selfX[:, :])
            iotac_f = const.tile([P, 1], dt.float32)
            nc.gpsimd.iota(iotac_f[:], pattern=[[0, 1]], base=0,
                           channel_multiplier=1,
                           allow_small_or_imprecise_dtypes=True)
            iota_col = const.tile([P, 1], dt.bfloat16)
            nc.vector.tensor_copy(iota_col[:], iotac_f[:])
            _source_side(nc, tc, ctx, srcrel,
                         lambda s: selfX_sb[:, s * P:(s + 1) * P],
                         m_out, T_out, DIM_IN, iota_col[:])
    nc.compile()
    return nc


def _build_p2(T_in, T_out):
    """P2: layer-1 destination side + dense + layer-2 pre-transforms +
    layer-2 source-side messages."""
    dt = mybir.dt
    NCHP_IN, NCHP_OUT = _nchp(T_in), _nchp(T_out)
    nc = bacc.Bacc("TRN2", target_bir_lowering=False, debug=False,
                   enable_asserts=False, num_devices=N_CORES)
    m_in = nc.dram_tensor("m_in", [P, NCHP_IN, DIM_IN], dt.bfloat16,
                          kind="ExternalInput").ap()
    selfT = nc.dram_tensor("selfT", [P, NPAD_CORE], dt.bfloat16,
                           kind="ExternalInput").ap()
    srcrel = nc.dram_tensor("srcrel", [1, NCHP_OUT * P], dt.bfloat16,
                            kind="ExternalInput").ap()
    w1l = nc.dram_tensor("w1l", [P, DIM_H], dt.bfloat16, kind="ExternalInput").ap()
    w1r = nc.dram_tensor("w1r", [P, DIM_H], dt.bfloat16, kind="ExternalInput").ap()
    b1 = nc.dram_tensor("b1", [P, 2], dt.float32, kind="ExternalInput").ap()
    w2l = nc.dram_tensor("w2l", [P, 2 * DIM_OUT], dt.bfloat16, kind="ExternalInput").ap()
    w2r = nc.dram_tensor("w2r", [P, 2 * DIM_OUT], dt.bfloat16, kind="ExternalInput").ap()
    b2 = nc.dram_tensor("b2", [1, DIM_OUT], dt.bfloat16, kind="ExternalInput").ap()
    mz_out = nc.dram_tensor("mz_out", [P, NCHP_OUT, DIM_OUT], dt.bfloat16,
                            kind="ExternalOutput").ap()
    s_out = nc.dram_tensor("s_out", [NPAD_CORE, DIM_OUT], dt.float32,
                           kind="ExternalOutput").ap()
    with tile.TileContext(nc) as tc:
        with ExitStack() as ctx:
            const = ctx.enter_context(tc.tile_pool(name="const", bufs=1))
            work = ctx.enter_context(tc.tile_pool(name="work", bufs=3))
            outp = ctx.enter_context(tc.tile_pool(name="outp", bufs=4))
            psA = ctx.enter_context(tc.tile_pool(name="psA", bufs=2, space="PSUM"))
            psT = ctx.enter_context(tc.tile_pool(name="psT", bufs=1, space="PSUM"))
            psH = ctx.enter_context(tc.tile_pool(name="psH", bufs=1, space="PSUM"))
            psZ = ctx.enter_context(tc.tile_pool(name="psZ", bufs=2, space="PSUM"))

            sb = _load_basics(nc, const, T_in)
            w1l_sb = const.tile([P, DIM_H], dt.bfloat16)
            nc.sync.dma_start(w1l_sb[:], w1l[:, :])
            w1r_sb = const.tile([P, DIM_H], dt.bfloat16)
            nc.sync.dma_start(w1r_sb[:], w1r[:, :])
            b1_sb = const.tile([P, 2], dt.float32)
            nc.sync.dma_start(b1_sb[:], b1[:, :])
            w2l_sb = const.tile([P, 2 * DIM_OUT], dt.bfloat16)
            nc.sync.dma_start(w2l_sb[:], w2l[:, :])
            w2r_sb = const.tile([P, 2 * DIM_OUT], dt.bfloat16)
            nc.sync.dma_start(w2r_sb[:], w2r[:, :])
            b2_sb = const.tile([1, DIM_OUT], dt.bfloat16)
            nc.sync.dma_start(b2_sb[:], b2[:, :])
            self_sb = const.tile([P, NPAD_CORE], dt.bfloat16)
            nc.sync.dma_start(self_sb[:], selfT[:, :])
            ident = const.tile([P, P], dt.bfloat16)
            make_identity(nc, ident[:])
            ones1 = const.tile([1, P], dt.bfloat16)
            nc.gpsimd.memset(ones1[:], 1.0)
            zall = const.tile([P, TILES_PER_CORE, DIM_OUT], dt.bfloat16)

            ensure = _dest_stream(nc, tc, ctx, m_in, sb["dst"], sb["iota_big"][:],
                                  T_in, DIM_IN)

            for t in range(TILES_PER_CORE):
                agg_ps = psA.tile([P, DIM_IN], dt.float32)
                for j in range(T_in):
                    msgs, S, slot = ensure(t * T_in + j)
                    nc.tensor.matmul(
                        out=agg_ps[:],
                        lhsT=S[:, slot * P:(slot + 1) * P],
                        rhs=msgs[:, slot, :],
                        start=(j == 0), stop=(j == T_in - 1))
                agg_sb = work.tile([P, DIM_IN], dt.bfloat16)
                nc.scalar.mul(agg_sb[:], agg_ps[:], sb["recip"][:, t:t + 1])
                tp = psT.tile([P, P], dt.bfloat16)
                nc.tensor.transpose(out=tp[:], in_=agg_sb[:], identity=ident[:])
                aggT_sb = work.tile([P, P], dt.bfloat16)
                nc.vector.tensor_copy(aggT_sb[:], tp[:])
                hT_sb = work.tile([P, 2, P], dt.bfloat16)
                for so in range(2):
                    h_ps = psH.tile([P, P], dt.float32)
                    nc.tensor.matmul(
                        out=h_ps[:], lhsT=w1l_sb[:, so * P:(so + 1) * P],
                        rhs=aggT_sb[:], start=True, stop=False)
                    nc.tensor.matmul(
                        out=h_ps[:], lhsT=w1r_sb[:, so * P:(so + 1) * P],
                        rhs=self_sb[:, t * P:(t + 1) * P],
                        start=False, stop=True)
                    nc.scalar.activation(
                        hT_sb[:, so, :], h_ps[:],
                        mybir.ActivationFunctionType.Relu,
                        bias=b1_sb[:, so:so + 1], scale=1.0)
                z_ps = psZ.tile([P, DIM_OUT], dt.float32, name="zs_ps")
                for si in range(2):
                    nc.tensor.matmul(
                        out=z_ps[:], lhsT=hT_sb[:, si, :],
                        rhs=w2l_sb[:, si * DIM_OUT:(si + 1) * DIM_OUT],
                        start=(si == 0), stop=(si == 1))
                nc.scalar.mul(zall[:, t, :], z_ps[:], 1.0)
                s_ps = psZ.tile([P, DIM_OUT], dt.float32, name="zs_ps")
                for si in range(2):
                    nc.tensor.matmul(
                        out=s_ps[:], lhsT=hT_sb[:, si, :],
                        rhs=w2r_sb[:, si * DIM_OUT:(si + 1) * DIM_OUT],
                        start=(si == 0), stop=False)
                nc.tensor.matmul(
                    out=s_ps[:], lhsT=ones1[:], rhs=b2_sb[:],
                    start=False, stop=True)
                s_sb = outp.tile([P, DIM_OUT], dt.float32)
                nc.scalar.mul(s_sb[:], s_ps[:], 1.0)
                nc.sync.dma_start(s_out[t * P:(t + 1) * P, :], s_sb[:])

            # layer-2 source side: z messages from SBUF-resident zall
            _source_side(nc, tc, ctx, srcrel,
                         lambda s: zall[:, s, :],
                         mz_out, T_out, DIM_OUT, sb["iota_col"][:],
                         prefix="z")
    nc.compile()
    return nc


def _build_p3(T_in):
    """P3: layer-2 destination side."""
    dt = mybir.dt
    NCHP_IN = _nchp(T_in)
    nc = bacc.Bacc("TRN2", target_bir_lowering=False, debug=False,
                   enable_asserts=False, num_devices=N_CORES)
    m_in = nc.dram_tensor("m_in", [P, NCHP_IN, DIM_OUT], dt.bfloat16,
                          kind="ExternalInput").ap()
    s_in = nc.dram_tensor("s_in", [NPAD_CORE, DIM_OUT], dt.float32,
                          kind="ExternalInput").ap()
    out = nc.dram_tensor("out", [NPAD_CORE, DIM_OUT], dt.float32,
                         kind="ExternalOutput").ap()
    with tile.TileContext(nc) as tc:
        with ExitStack() as ctx:
            const = ctx.enter_context(tc.tile_pool(name="const", bufs=1))
            sload = ctx.enter_context(tc.tile_pool(name="sload", bufs=3))
            outp = ctx.enter_context(tc.tile_pool(name="outp", bufs=4))
            psA = ctx.enter_context(tc.tile_pool(name="psA", bufs=2, space="PSUM"))
            sb = _load_basics(nc, const, T_in)
            ensure = _dest_stream(nc, tc, ctx, m_in, sb["dst"], sb["iota_big"][:],
                                  T_in, DIM_OUT)
            for t in range(TILES_PER_CORE):
                agg_ps = psA.tile([P, DIM_OUT], dt.float32)
                for j in range(T_in):
                    msgs, S, slot = ensure(t * T_in + j)
                    nc.tensor.matmul(
                        out=agg_ps[:],
                        lhsT=S[:, slot * P:(slot + 1) * P],
                        rhs=msgs[:, slot, :],
                        start=(j == 0), stop=(j == T_in - 1))
                s_tile = sload.tile([P, DIM_OUT], dt.float32)
                nc.sync.dma_start(s_tile[:], s_in[t * P:(t + 1) * P, :])
                agg_sb = outp.tile([P, DIM_OUT], dt.float32)
                nc.scalar.mul(agg_sb[:], agg_ps[:], sb["recip"][:, t:t + 1])
                o_sb = outp.tile([P, DIM_OUT], dt.float32)
                nc.vector.tensor_add(o_sb[:], agg_sb[:], s_tile[:])
                nc.sync.dma_start(out[t * P:(t + 1) * P, :], o_sb[:])
    nc.compile()
    return nc


_PROG_CACHE = {}


def _get_programs(T_in, T_out):
    key = (T_in, T_out)
    if key not in _PROG_CACHE:
        _PROG_CACHE[key] = (_build_p1(T_out), _build_p2(T_in, T_out),
                            _build_p3(T_in))
    return _PROG_CACHE[key]


def _permute(m_results, gpos1, dpos2, T_in, fdim):
    """Host all-to-all: source-order message rows -> per-destination-core
    tile-major arrays [128, NCHP_IN, fdim] (transposed-chunk layout)."""
    NCHP_IN = _nchp(T_in)
    flat = np.concatenate(
        [np.ascontiguousarray(m.transpose(1, 0, 2)).reshape(-1, fdim)
         for m in m_results], axis=0)
    dcore, fpos2 = dpos2
    outs = []
    for d in range(N_CORES):
        m = dcore == d
        arr = np.zeros((NCHP_IN * P, fdim), flat.dtype)
        arr[fpos2[m]] = flat[gpos1[m]]
        outs.append(np.ascontiguousarray(
            arr.reshape(NCHP_IN, P, fdim).transpose(1, 0, 2)))
    return outs


def kernel(x, edge_index, W1l, W1r, b1, W2l, W2r, b2):
    global LAST_RESULTS
    LAST_RESULTS = []
    x = np.asarray(x, np.float32)
    src = np.asarray(edge_index[0], np.int64)
    dst = np.asarray(edge_index[1], np.int64)

    deg_in = np.bincount(dst, minlength=N_NODES)
    deg_out = np.bincount(src, minlength=N_NODES)
    tile_of, slot_of, T_in, T_out = _partition_nodes(deg_in, deg_out)
    gpos1, dpos2, srcrel, dstrel = _build_layout(
        src, dst, tile_of, slot_of, T_in, T_out)

    p1, p2, p3 = _get_programs(T_in, T_out)

    trace = bool(int(__import__("os").environ.get("BASS_TRACE", "0") or 0))
    tkw = dict(trace=True, tmpdir=None) if trace else {}

    x_bf = x.astype(BF16)
    deg_cols, selfTs, selfXs, node_lists, local_lists = [], [], [], [], []
    for c in range(N_CORES):
        tiles = np.arange(c * TILES_PER_CORE, (c + 1) * TILES_PER_CORE)
        mask = np.isin(tile_of, tiles)
        nodes = np.nonzero(mask)[0]
        local_tile = tile_of[nodes] - c * TILES_PER_CORE
        local = local_tile * P + slot_of[nodes]
        dcol = np.zeros((P, TILES_PER_CORE), np.float32)
        dcol[slot_of[nodes], local_tile] = deg_in[nodes]
        sN = np.zeros((NPAD_CORE, DIM_IN), BF16)
        sN[local] = x_bf[nodes]
        deg_cols.append(dcol)
        selfTs.append(np.ascontiguousarray(sN.T))
        selfXs.append(np.ascontiguousarray(
            sN.reshape(TILES_PER_CORE, P, DIM_IN).transpose(1, 0, 2)
            .reshape(P, NPAD_CORE)))
        node_lists.append(nodes)
        local_lists.append(local)

    w1l_p = np.ascontiguousarray(np.asarray(W1l, np.float32).T).astype(BF16)
    w1r_p = np.ascontiguousarray(np.asarray(W1r, np.float32).T).astype(BF16)
    b1_np = np.asarray(b1, np.float32)
    b1_p = np.stack([b1_np[:P], b1_np[P:]], axis=1).astype(np.float32)
    w2l_np = np.asarray(W2l, np.float32)
    w2r_np = np.asarray(W2r, np.float32)
    w2l_p = np.ascontiguousarray(np.hstack([w2l_np.T[:P, :], w2l_np.T[P:, :]])).astype(BF16)
    w2r_p = np.ascontiguousarray(np.hstack([w2r_np.T[:P, :], w2r_np.T[P:, :]])).astype(BF16)
    b2_p = np.asarray(b2, np.float32)[None, :].astype(BF16)

    r1 = _run_spmd_retry(p1, [
        {"selfX": selfXs[c], "srcrel": srcrel[c]} for c in range(N_CORES)], **tkw)
    LAST_RESULTS.append(r1)

    m1 = _permute([r1.results[c]["m_out"] for c in range(N_CORES)],
                  gpos1, dpos2, T_in, DIM_IN)

    r2 = _run_spmd_retry(p2, [
        {"m_in": m1[c], "selfT": selfTs[c], "srcrel": srcrel[c],
         "dstrel": dstrel[c], "deg_col": deg_cols[c],
         "w1l": w1l_p, "w1r": w1r_p, "b1": b1_p,
         "w2l": w2l_p, "w2r": w2r_p, "b2": b2_p} for c in range(N_CORES)], **tkw)
    LAST_RESULTS.append(r2)

    m2 = _permute([r2.results[c]["mz_out"] for c in range(N_CORES)],
                  gpos1, dpos2, T_in, DIM_OUT)

    r3 = _run_spmd_retry(p3, [
        {"m_in": m2[c], "s_in": r2.results[c]["s_out"],
         "dstrel": dstrel[c], "deg_col": deg_cols[c]} for c in range(N_CORES)], **tkw)
    LAST_RESULTS.append(r3)

    out = np.zeros((N_NODES, DIM_OUT), np.float32)
    for c in range(N_CORES):
        out[node_lists[c]] = r3.results[c]["out"][local_lists[c]]
    return out


# revision 14
# speedup vs baseline: 1.4979x; 1.2681x over previous
"""GraphSAGE via all-to-all neighbor-message exchange (v3).

Pipeline (3 SPMD programs, host relays the all-to-all between them):

P1  source side, layer 1: each core forms the per-edge messages x[src] for
    its OWN nodes' outgoing edges with one-hot PE matmuls from SBUF-resident
    x (no DMA descriptors per edge) and writes them contiguously, grouped by
    source tile.
H1  host all-to-all: the per-edge message rows (device-produced) are
    permuted from source order into each destination core's tile-major
    order. Pure bijective re-layout of device data; no arithmetic.
P2  destination side layer 1 + source side layer 2: contiguous reads of the
    permuted messages, S^T matmul segment-mean, dense h = ...W1..., then
    z = h@W2l^T and s = h@W2r^T + b2, and the layer-2 source-side messages
    z[src] (same one-hot machinery, z stays in SBUF).
H2  host all-to-all of the z-messages (same permutation).
P3  destination side layer 2: segment-mean of z-messages + s.

The one-hot matrices: destination side S[e, n] = (dst_slot[e] == n) is
built with edge-on-partition broadcast + iota (as in v2). Source side
needs slot-on-partition Sel[p, e] = (src_slot[e] == p): the slot sequence
is broadcast-DMA'd across partitions and compared against a partition-index
iota column.
"""
import sys
from contextlib import ExitStack

import numpy as np

for _p in ("/opt/trn_rl_repo",):
    if _p not in sys.path:
        sys.path.insert(0, _p)

import concourse.bass as bass
import concourse.tile as tile
from concourse import bacc, mybir
from concourse.bass_utils import run_bass_kernel_spmd
from concourse.masks import make_identity

try:
    import ml_dtypes
    BF16 = ml_dtypes.bfloat16
except ImportError:  # pragma: no cover
    import jax.numpy as jnp
    BF16 = jnp.bfloat16

def _ensure_axon_hooks():
    """run_bass_kernel_spmd(trace=True) imports antenv.axon_hooks, which this
    image lacks; install a ctypes-backed hook so tracing works (or degrades
    to a no-op instead of an ImportError)."""
    try:
        import antenv.axon_hooks  # noqa: F401
        return
    except ImportError:
        pass
    import contextlib
    import ctypes
    import types

    def _make_hook():
        try:
            lib = ctypes.CDLL("/opt/axon/libaxon_pjrt.so")
        except OSError:
            return None
        if not hasattr(lib, "axon_start_nrt_profile"):
            return None
        lib.axon_start_nrt_profile.argtypes = [ctypes.POINTER(ctypes.c_int64), ctypes.c_size_t]
        lib.axon_start_nrt_profile.restype = ctypes.c_int64
        lib.axon_stop_nrt_profile.argtypes = [ctypes.c_char_p]
        lib.axon_stop_nrt_profile.restype = ctypes.c_int64

        @contextlib.contextmanager
        def _hook(output_dir, device_ids):
            import jax
            jax.devices()
            if device_ids:
                ids = (ctypes.c_int64 * len(device_ids))(*device_ids)
                rc = lib.axon_start_nrt_profile(ids, len(device_ids))
            else:
                rc = lib.axon_start_nrt_profile(None, 0)
            if rc != 0:
                raise RuntimeError(f"axon_start_nrt_profile rc={rc}")
            try:
                yield
            finally:
                lib.axon_stop_nrt_profile(str(output_dir).encode())

        return _hook

    hook = _make_hook()
    mod = types.ModuleType("antenv.axon_hooks")
    mod.get_axon_ntff_profile_hook = lambda: hook
    mod.set_axon_ntff_profile_hook = lambda h: None
    import antenv
    antenv.axon_hooks = mod
    sys.modules["antenv.axon_hooks"] = mod


_ensure_axon_hooks()

N_NODES = 50000
N_EDGES = 800000
DIM_IN, DIM_H, DIM_OUT = 128, 256, 64
N_CORES = 8
P = 128
TILES_PER_CORE = 49
N_TILES = N_CORES * TILES_PER_CORE       # 392
NPAD_CORE = TILES_PER_CORE * P           # 6272
PAD_SLOT = 200.0
QCH = 8                                  # chunks per message group (DMA unit)

LAST_RESULTS = []


def _run_spmd_retry(nc, in_maps, **kw):
    import time
    try:
        return run_bass_kernel_spmd(nc, in_maps, core_ids=list(range(N_CORES)), **kw)
    except Exception:
        time.sleep(15)
        return run_bass_kernel_spmd(nc, in_maps, core_ids=list(range(N_CORES)), **kw)


def _nchp(T):
    n = TILES_PER_CORE * T
    return (n + QCH - 1) // QCH * QCH


def _partition_nodes(deg_in, deg_out):
    """Greedy 2D-balanced packing of nodes into tiles of <=128, balancing
    per-tile in-degree and out-degree sums."""
    deg = deg_in + deg_out
    order = np.argsort(-deg, kind="stable")
    sumsI = np.zeros(N_TILES, np.float64)
    sumsO = np.zeros(N_TILES, np.float64)
    counts = np.zeros(N_TILES, np.int64)
    tile_of = np.empty(N_NODES, np.int64)
    slot_of = np.empty(N_NODES, np.int64)
    capI = float(deg_in.sum()) / N_TILES
    capO = float(deg_out.sum()) / N_TILES
    for node in order:
        dI, dO = deg_in[node], deg_out[node]
        score = np.maximum((sumsI + dI) / capI, (sumsO + dO) / capO)
        score[counts >= P] = np.inf
        t = int(np.argmin(score))
        tile_of[node] = t
        slot_of[node] = counts[t]
        counts[t] += 1
        sumsI[t] += dI
        sumsO[t] += dO
    T_in = int(np.ceil(sumsI.max() / P))
    T_out = int(np.ceil(sumsO.max() / P))
    return tile_of, slot_of, T_in, T_out


def _rank_in_groups(keys, n_groups):
    """For each element, its rank within its key-group (stable)."""
    order = np.argsort(keys, kind="stable")
    counts = np.bincount(keys, minlength=n_groups)
    starts = np.concatenate([[0], np.cumsum(counts)[:-1]])
    rank_sorted = np.arange(len(keys)) - np.repeat(starts, counts)
    rank = np.empty(len(keys), np.int64)
    rank[order] = rank_sorted
    return rank


def _build_layout(src, dst, tile_of, slot_of, T_in, T_out):
    """Edge placement metadata.

    Returns:
      gpos1[e]: global flat row of edge e in the concatenated source-order
                message array (core-major, [NCHP_OUT*128] rows per core).
      dpos2[e]: (dst_core, flat row) of edge e in the destination-order
                message array ([NCHP_IN*128] rows per core).
      srcrel:   per-core [1, NCHP_OUT*128] bf16 slot sequence (PAD on pads).
      dstrel:   per-core [P, NCHP_IN] bf16 dst-slot per edge lane.
    """
    NCHP_IN, NCHP_OUT = _nchp(T_in), _nchp(T_out)
    stile, sslot = tile_of[src], slot_of[src]
    dtile, dslot = tile_of[dst], slot_of[dst]

    rank1 = _rank_in_groups(stile, N_TILES)
    fpos1 = (stile % TILES_PER_CORE) * T_out * P + rank1
    gpos1 = (stile // TILES_PER_CORE) * (NCHP_OUT * P) + fpos1

    rank2 = _rank_in_groups(dtile, N_TILES)
    fpos2 = (dtile % TILES_PER_CORE) * T_in * P + rank2
    dcore = dtile // TILES_PER_CORE

    srcrel, dstrel = [], []
    for c in range(N_CORES):
        m = (stile // TILES_PER_CORE) == c
        row = np.full(NCHP_OUT * P, PAD_SLOT, np.float32)
        row[fpos1[m]] = sslot[m]
        srcrel.append(np.ascontiguousarray(row[None, :].astype(BF16)))
        m2 = dcore == c
        flat = np.full(NCHP_IN * P, PAD_SLOT, np.float32)
        flat[fpos2[m2]] = dslot[m2]
        dstrel.append(np.ascontiguousarray(
            flat.reshape(NCHP_IN, P).T.astype(BF16)))
    return gpos1, (dcore, fpos2), srcrel, dstrel


# ---------------------------------------------------------------- programs

def _source_side(nc, tc, ctx, srcrel_d, rhs_fn, m_out, T_out,
                 fdim, iota_col, prefix="", ps_bufs=2):
    """Emit the source-side message formation: for each QCH-chunk group,
    broadcast-load the slot sequence, build Sel[p, e] = (p == slot[e]),
    matmul per chunk against the source tile's rows, copy the group's PSUM
    to bf16 SBUF (alternating scalar/vector), and DMA out."""
    dt = mybir.dt
    NCHP_OUT = _nchp(T_out)
    repp = ctx.enter_context(tc.tile_pool(name=f"{prefix}repp", bufs=4))
    selp = ctx.enter_context(tc.tile_pool(name=f"{prefix}selp", bufs=4))
    mb = ctx.enter_context(tc.tile_pool(name=f"{prefix}mb", bufs=4))
    psM = ctx.enter_context(tc.tile_pool(name=f"{prefix}psM", bufs=ps_bufs,
                                         space="PSUM"))
    for g in range(NCHP_OUT // QCH):
        rep = repp.tile([P, QCH * P], dt.bfloat16, name=f"{prefix}rep")
        nc.sync.dma_start(
            rep[:], srcrel_d[0:1, g * QCH * P:(g + 1) * QCH * P]
            .to_broadcast([P, QCH * P]))
        sel = selp.tile([P, QCH * P], dt.bfloat16, name=f"{prefix}sel")
        nc.vector.tensor_tensor(
            out=sel[:], in0=iota_col.to_broadcast([P, QCH * P]), in1=rep[:],
            op=mybir.AluOpType.is_equal)
        ps = psM.tile([P, QCH, fdim], dt.float32, name=f"{prefix}mps")
        for jj in range(QCH):
            j = g * QCH + jj
            s = min(j // T_out, TILES_PER_CORE - 1)
            nc.tensor.matmul(
                out=ps[:, jj, :],
                lhsT=sel[:, jj * P:(jj + 1) * P],
                rhs=rhs_fn(s),
                start=True, stop=True)
        mt = mb.tile([P, QCH, fdim], dt.bfloat16, name=f"{prefix}mt")
        if g % 2 == 0:
            nc.vector.tensor_copy(mt[:], ps[:])
        else:
            nc.scalar.mul(mt[:], ps[:], 1.0)
        nc.sync.dma_start(m_out[:, g * QCH:(g + 1) * QCH, :], mt[:])


def _dest_stream(nc, tc, ctx, m_in, dst_sb, iota_big, T_in, fdim, prefix=""):
    """Streaming loader for the destination side: groups of QCH chunks are
    DMA'd from the permuted message array and their S blocks built.
    Returns ensure(chunk) -> (msgs_tile, S_tile, slot)."""
    dt = mybir.dt
    mp = ctx.enter_context(tc.tile_pool(name=f"{prefix}mp", bufs=6))
    sp = ctx.enter_context(tc.tile_pool(name=f"{prefix}sp", bufs=6))
    state = {"next": 0, "msgs": {}, "S": {}}

    def issue(g):
        msgs = mp.tile([P, QCH, fdim], dt.bfloat16, name=f"{prefix}msgs")
        nc.sync.dma_start(msgs[:], m_in[:, g * QCH:(g + 1) * QCH, :])
        S = sp.tile([P, QCH * P], dt.bfloat16, name=f"{prefix}S")
        try:
            nc.vector.tensor_tensor(
                out=S[:],
                in0=dst_sb[:, g * QCH:(g + 1) * QCH, None].to_broadcast(
                    [P, QCH, P]),
                in1=iota_big[:],
                op=mybir.AluOpType.is_equal)
        except Exception:
            for j in range(QCH):
                nc.vector.tensor_tensor(
                    out=S[:, j * P:(j + 1) * P],
                    in0=dst_sb[:, g * QCH + j:g * QCH + j + 1].to_broadcast([P, P]),
                    in1=iota_big[:, :P],
                    op=mybir.AluOpType.is_equal)
        state["msgs"][g] = msgs
        state["S"][g] = S

    def ensure(chunk):
        g = chunk // QCH
        while state["next"] <= g:
            issue(state["next"])
            state["next"] += 1
        return state["msgs"][g], state["S"][g], chunk % QCH

    return ensure


def _load_basics(nc, const, T_in):
    """iota tiles, dstrel, deg/recip. Returns dict."""
    dt = mybir.dt
    NCHP_IN = _nchp(T_in)
    dstrel = nc.dram_tensor("dstrel", [P, NCHP_IN], dt.bfloat16,
                            kind="ExternalInput").ap()
    deg_col = nc.dram_tensor("deg_col", [P, TILES_PER_CORE], dt.float32,
                             kind="ExternalInput").ap()
    s = {}
    dst_sb = const.tile([P, NCHP_IN], dt.bfloat16)
    nc.sync.dma_start(dst_sb[:], dstrel[:, :])
    s["dst"] = dst_sb
    deg_sb = const.tile([P, TILES_PER_CORE], dt.float32)
    nc.sync.dma_start(deg_sb[:], deg_col[:, :])
    iota_f = const.tile([P, P], dt.float32)
    nc.gpsimd.iota(iota_f[:], pattern=[[1, P]], base=0, channel_multiplier=0,
                   allow_small_or_imprecise_dtypes=True)
    iota_sm = const.tile([P, P], dt.bfloat16)
    nc.vector.tensor_copy(iota_sm[:], iota_f[:])
    iota_big = const.tile([P, QCH * P], dt.bfloat16)
    for _j in range(QCH):
        nc.vector.tensor_copy(iota_big[:, _j * P:(_j + 1) * P], iota_sm[:])
    s["iota_big"] = iota_big
    iotac_f = const.tile([P, 1], dt.float32)
    nc.gpsimd.iota(iotac_f[:], pattern=[[0, 1]], base=0, channel_multiplier=1,
                   allow_small_or_imprecise_dtypes=True)
    iota_col = const.tile([P, 1], dt.bfloat16)
    nc.vector.tensor_copy(iota_col[:], iotac_f[:])
    s["iota_col"] = iota_col
    recip = const.tile([P, TILES_PER_CORE], dt.float32)
    nc.vector.tensor_scalar_max(recip[:], deg_sb[:], 1.0)
    nc.vector.reciprocal(recip[:], recip[:])
    s["recip"] = recip
    return s


def _build_p1(T_out):
    """P1: layer-1 source-side messages."""
    dt = mybir.dt
    NCHP_OUT = _nchp(T_out)
    nc = bacc.Bacc("TRN2", target_bir_lowering=False, debug=False,
                   enable_asserts=False, num_devices=N_CORES)
    selfX = nc.dram_tensor("selfX", [P, NPAD_CORE], dt.bfloat16,
                           kind="ExternalInput").ap()
    srcrel = nc.dram_tensor("srcrel", [1, NCHP_OUT * P], dt.bfloat16,
                            kind="ExternalInput").ap()
    m_out = nc.dram_tensor("m_out", [P, NCHP_OUT, DIM_IN], dt.bfloat16,
                           kind="ExternalOutput").ap()
    with tile.TileContext(nc) as tc:
        with ExitStack() as ctx:
            const = ctx.enter_context(tc.tile_pool(name="const", bufs=1))
            selfX_sb = const.tile([P, NPAD_CORE], dt.bfloat16)
            nc.sync.dma_start(selfX_sb[:], selfX[:, :])
            iotac_f = const.tile([P, 1], dt.float32)
            nc.gpsimd.iota(iotac_f[:], pattern=[[0, 1]], base=0,
                           channel_multiplier=1,
                           allow_small_or_imprecise_dtypes=True)
            iota_col = const.tile([P, 1], dt.bfloat16)
            nc.vector.tensor_copy(iota_col[:], iotac_f[:])
            _source_side(nc, tc, ctx, srcrel,
                         lambda s: selfX_sb[:, s * P:(s + 1) * P],
                         m_out, T_out, DIM_IN, iota_col[:], ps_bufs=3)
    nc.compile()
    return nc


def _build_p2(T_in, T_out):
    """P2: layer-1 destination side + dense + layer-2 pre-transforms +
    layer-2 source-side messages."""
    dt = mybir.dt
    NCHP_IN, NCHP_OUT = _nchp(T_in), _nchp(T_out)
    nc = bacc.Bacc("TRN2", target_bir_lowering=False, debug=False,
                   enable_asserts=False, num_devices=N_CORES)
    m_in = nc.dram_tensor("m_in", [P, NCHP_IN, DIM_IN], dt.bfloat16,
                          kind="ExternalInput").ap()
    selfT = nc.dram_tensor("selfT", [P, NPAD_CORE], dt.bfloat16,
                           kind="ExternalInput").ap()
    srcrel = nc.dram_tensor("srcrel", [1, NCHP_OUT * P], dt.bfloat16,
                            kind="ExternalInput").ap()
    w1l = nc.dram_tensor("w1l", [P, DIM_H], dt.bfloat16, kind="ExternalInput").ap()
    w1r = nc.dram_tensor("w1r", [P, DIM_H], dt.bfloat16, kind="ExternalInput").ap()
    b1 = nc.dram_tensor("b1", [P, 2], dt.float32, kind="ExternalInput").ap()
    w2l = nc.dram_tensor("w2l", [P, 2 * DIM_OUT], dt.bfloat16, kind="ExternalInput").ap()
    w2r = nc.dram_tensor("w2r", [P, 2 * DIM_OUT], dt.bfloat16, kind="ExternalInput").ap()
    b2 = nc.dram_tensor("b2", [1, DIM_OUT], dt.bfloat16, kind="ExternalInput").ap()
    mz_out = nc.dram_tensor("mz_out", [P, NCHP_OUT, DIM_OUT], dt.bfloat16,
                            kind="ExternalOutput").ap()
    s_out = nc.dram_tensor("s_out", [NPAD_CORE, DIM_OUT], dt.float32,
                           kind="ExternalOutput").ap()
    with tile.TileContext(nc) as tc:
        with ExitStack() as ctx:
            const = ctx.enter_context(tc.tile_pool(name="const", bufs=1))
            work = ctx.enter_context(tc.tile_pool(name="work", bufs=3))
            outp = ctx.enter_context(tc.tile_pool(name="outp", bufs=4))
            psA = ctx.enter_context(tc.tile_pool(name="psA", bufs=2, space="PSUM"))
            psT = ctx.enter_context(tc.tile_pool(name="psT", bufs=1, space="PSUM"))
            psH = ctx.enter_context(tc.tile_pool(name="psH", bufs=1, space="PSUM"))
            psZ = ctx.enter_context(tc.tile_pool(name="psZ", bufs=2, space="PSUM"))

            sb = _load_basics(nc, const, T_in)
            w1l_sb = const.tile([P, DIM_H], dt.bfloat16)
            nc.sync.dma_start(w1l_sb[:], w1l[:, :])
            w1r_sb = const.tile([P, DIM_H], dt.bfloat16)
            nc.sync.dma_start(w1r_sb[:], w1r[:, :])
            b1_sb = const.tile([P, 2], dt.float32)
            nc.sync.dma_start(b1_sb[:], b1[:, :])
            w2l_sb = const.tile([P, 2 * DIM_OUT], dt.bfloat16)
            nc.sync.dma_start(w2l_sb[:], w2l[:, :])
            w2r_sb = const.tile([P, 2 * DIM_OUT], dt.bfloat16)
            nc.sync.dma_start(w2r_sb[:], w2r[:, :])
            b2_sb = const.tile([1, DIM_OUT], dt.bfloat16)
            nc.sync.dma_start(b2_sb[:], b2[:, :])
            self_sb = const.tile([P, NPAD_CORE], dt.bfloat16)
            nc.sync.dma_start(self_sb[:], selfT[:, :])
            ident = const.tile([P, P], dt.bfloat16)
            make_identity(nc, ident[:])
            ones1 = const.tile([1, P], dt.bfloat16)
            nc.gpsimd.memset(ones1[:], 1.0)
            zall = const.tile([P, TILES_PER_CORE, DIM_OUT], dt.bfloat16)

            ensure = _dest_stream(nc, tc, ctx, m_in, sb["dst"], sb["iota_big"][:],
                                  T_in, DIM_IN)

            for t in range(TILES_PER_CORE):
                agg_ps = psA.tile([P, DIM_IN], dt.float32)
                for j in range(T_in):
                    msgs, S, slot = ensure(t * T_in + j)
                    nc.tensor.matmul(
                        out=agg_ps[:],
                        lhsT=S[:, slot * P:(slot + 1) * P],
                        rhs=msgs[:, slot, :],
                        start=(j == 0), stop=(j == T_in - 1))
                agg_sb = work.tile([P, DIM_IN], dt.bfloat16)
                nc.scalar.mul(agg_sb[:], agg_ps[:], sb["recip"][:, t:t + 1])
                tp = psT.tile([P, P], dt.bfloat16)
                nc.tensor.transpose(out=tp[:], in_=agg_sb[:], identity=ident[:])
                aggT_sb = work.tile([P, P], dt.bfloat16)
                nc.vector.tensor_copy(aggT_sb[:], tp[:])
                hT_sb = work.tile([P, 2, P], dt.bfloat16)
                for so in range(2):
                    h_ps = psH.tile([P, P], dt.float32)
                    nc.tensor.matmul(
                        out=h_ps[:], lhsT=w1l_sb[:, so * P:(so + 1) * P],
                        rhs=aggT_sb[:], start=True, stop=False)
                    nc.tensor.matmul(
                        out=h_ps[:], lhsT=w1r_sb[:, so * P:(so + 1) * P],
                        rhs=self_sb[:, t * P:(t + 1) * P],
                        start=False, stop=True)
                    nc.scalar.activation(
                        hT_sb[:, so, :], h_ps[:],
                        mybir.ActivationFunctionType.Relu,
                        bias=b1_sb[:, so:so + 1], scale=1.0)
                z_ps = psZ.tile([P, DIM_OUT], dt.float32, name="zs_ps")
                for si in range(2):
                    nc.tensor.matmul(
                        out=z_ps[:], lhsT=hT_sb[:, si, :],
                        rhs=w2l_sb[:, si * DIM_OUT:(si + 1) * DIM_OUT],
                        start=(si == 0), stop=(si == 1))
                nc.scalar.mul(zall[:, t, :], z_ps[:], 1.0)
                s_ps = psZ.tile([P, DIM_OUT], dt.float32, name="zs_ps")
                for si in range(2):
                    nc.tensor.matmul(
                        out=s_ps[:], lhsT=hT_sb[:, si, :],
                        rhs=w2r_sb[:, si * DIM_OUT:(si + 1) * DIM_OUT],
                        start=(si == 0), stop=False)
                nc.tensor.matmul(
                    out=s_ps[:], lhsT=ones1[:], rhs=b2_sb[:],
                    start=False, stop=True)
                s_sb = outp.tile([P, DIM_OUT], dt.float32)
                nc.scalar.mul(s_sb[:], s_ps[:], 1.0)
                nc.sync.dma_start(s_out[t * P:(t + 1) * P, :], s_sb[:])

            # layer-2 source side: z messages from SBUF-resident zall
            _source_side(nc, tc, ctx, srcrel,
                         lambda s: zall[:, s, :],
                         mz_out, T_out, DIM_OUT, sb["iota_col"][:],
                         prefix="z")
    nc.compile()
    return nc


def _build_p3(T_in):
    """P3: layer-2 destination side."""
    dt = mybir.dt
    NCHP_IN = _nchp(T_in)
    nc = bacc.Bacc("TRN2", target_bir_lowering=False, debug=False,
                   enable_asserts=False, num_devices=N_CORES)
    m_in = nc.dram_tensor("m_in", [P, NCHP_IN, DIM_OUT], dt.bfloat16,
                          kind="ExternalInput").ap()
    s_in = nc.dram_tensor("s_in", [NPAD_CORE, DIM_OUT], dt.float32,
                          kind="ExternalInput").ap()
    out = nc.dram_tensor("out", [NPAD_CORE, DIM_OUT], dt.float32,
                         kind="ExternalOutput").ap()
    with tile.TileContext(nc) as tc:
        with ExitStack() as ctx:
            const = ctx.enter_context(tc.tile_pool(name="const", bufs=1))
            sload = ctx.enter_context(tc.tile_pool(name="sload", bufs=6))
            outp = ctx.enter_context(tc.tile_pool(name="outp", bufs=6))
            psA = ctx.enter_context(tc.tile_pool(name="psA", bufs=4, space="PSUM"))
            sb = _load_basics(nc, const, T_in)
            ensure = _dest_stream(nc, tc, ctx, m_in, sb["dst"], sb["iota_big"][:],
                                  T_in, DIM_OUT)
            for t in range(TILES_PER_CORE):
                agg_ps = psA.tile([P, DIM_OUT], dt.float32)
                for j in range(T_in):
                    msgs, S, slot = ensure(t * T_in + j)
                    nc.tensor.matmul(
                        out=agg_ps[:],
                        lhsT=S[:, slot * P:(slot + 1) * P],
                        rhs=msgs[:, slot, :],
                        start=(j == 0), stop=(j == T_in - 1))
                s_tile = sload.tile([P, DIM_OUT], dt.float32)
                nc.sync.dma_start(s_tile[:], s_in[t * P:(t + 1) * P, :])
                agg_sb = outp.tile([P, DIM_OUT], dt.float32)
                nc.scalar.mul(agg_sb[:], agg_ps[:], sb["recip"][:, t:t + 1])
                o_sb = outp.tile([P, DIM_OUT], dt.float32)
                nc.vector.tensor_add(o_sb[:], agg_sb[:], s_tile[:])
                nc.sync.dma_start(out[t * P:(t + 1) * P, :], o_sb[:])
    nc.compile()
    return nc


_PROG_CACHE = {}


def _get_programs(T_in, T_out):
    key = (T_in, T_out)
    if key not in _PROG_CACHE:
        _PROG_CACHE[key] = (_build_p1(T_out), _build_p2(T_in, T_out),
                            _build_p3(T_in))
    return _PROG_CACHE[key]


def _permute(m_results, gpos1, dpos2, T_in, fdim):
    """Host all-to-all: source-order message rows -> per-destination-core
    tile-major arrays [128, NCHP_IN, fdim] (transposed-chunk layout)."""
    NCHP_IN = _nchp(T_in)
    flat = np.concatenate(
        [np.ascontiguousarray(m.transpose(1, 0, 2)).reshape(-1, fdim)
         for m in m_results], axis=0)
    dcore, fpos2 = dpos2
    outs = []
    for d in range(N_CORES):
        m = dcore == d
        arr = np.zeros((NCHP_IN * P, fdim), flat.dtype)
        arr[fpos2[m]] = flat[gpos1[m]]
        outs.append(np.ascontiguousarray(
            arr.reshape(NCHP_IN, P, fdim).transpose(1, 0, 2)))
    return outs


def kernel(x, edge_index, W1l, W1r, b1, W2l, W2r, b2):
    global LAST_RESULTS
    LAST_RESULTS = []
    x = np.asarray(x, np.float32)
    src = np.asarray(edge_index[0], np.int64)
    dst = np.asarray(edge_index[1], np.int64)

    deg_in = np.bincount(dst, minlength=N_NODES)
    deg_out = np.bincount(src, minlength=N_NODES)
    tile_of, slot_of, T_in, T_out = _partition_nodes(deg_in, deg_out)
    gpos1, dpos2, srcrel, dstrel = _build_layout(
        src, dst, tile_of, slot_of, T_in, T_out)

    p1, p2, p3 = _get_programs(T_in, T_out)

    trace = bool(int(__import__("os").environ.get("BASS_TRACE", "0") or 0))
    tkw = dict(trace=True, tmpdir=None) if trace else {}

    x_bf = x.astype(BF16)
    deg_cols, selfTs, selfXs, node_lists, local_lists = [], [], [], [], []
    for c in range(N_CORES):
        tiles = np.arange(c * TILES_PER_CORE, (c + 1) * TILES_PER_CORE)
        mask = np.isin(tile_of, tiles)
        nodes = np.nonzero(mask)[0]
        local_tile = tile_of[nodes] - c * TILES_PER_CORE
        local = local_tile * P + slot_of[nodes]
        dcol = np.zeros((P, TILES_PER_CORE), np.float32)
        dcol[slot_of[nodes], local_tile] = deg_in[nodes]
        sN = np.zeros((NPAD_CORE, DIM_IN), BF16)
        sN[local] = x_bf[nodes]
        deg_cols.append(dcol)
        selfTs.append(np.ascontiguousarray(sN.T))
        selfXs.append(np.ascontiguousarray(
            sN.reshape(TILES_PER_CORE, P, DIM_IN).transpose(1, 0, 2)
            .reshape(P, NPAD_CORE)))
        node_lists.append(nodes)
        local_lists.append(local)

    w1l_p = np.ascontiguousarray(np.asarray(W1l, np.float32).T).astype(BF16)
    w1r_p = np.ascontiguousarray(np.asarray(W1r, np.float32).T).astype(BF16)
    b1_np = np.asarray(b1, np.float32)
    b1_p = np.stack([b1_np[:P], b1_np[P:]], axis=1).astype(np.float32)
    w2l_np = np.asarray(W2l, np.float32)
    w2r_np = np.asarray(W2r, np.float32)
    w2l_p = np.ascontiguousarray(np.hstack([w2l_np.T[:P, :], w2l_np.T[P:, :]])).astype(BF16)
    w2r_p = np.ascontiguousarray(np.hstack([w2r_np.T[:P, :], w2r_np.T[P:, :]])).astype(BF16)
    b2_p = np.asarray(b2, np.float32)[None, :].astype(BF16)

    r1 = _run_spmd_retry(p1, [
        {"selfX": selfXs[c], "srcrel": srcrel[c]} for c in range(N_CORES)], **tkw)
    LAST_RESULTS.append(r1)

    m1 = _permute([r1.results[c]["m_out"] for c in range(N_CORES)],
                  gpos1, dpos2, T_in, DIM_IN)

    r2 = _run_spmd_retry(p2, [
        {"m_in": m1[c], "selfT": selfTs[c], "srcrel": srcrel[c],
         "dstrel": dstrel[c], "deg_col": deg_cols[c],
         "w1l": w1l_p, "w1r": w1r_p, "b1": b1_p,
         "w2l": w2l_p, "w2r": w2r_p, "b2": b2_p} for c in range(N_CORES)], **tkw)
    LAST_RESULTS.append(r2)

    m2 = _permute([r2.results[c]["mz_out"] for c in range(N_CORES)],
                  gpos1, dpos2, T_in, DIM_OUT)

    r3 = _run_spmd_retry(p3, [
        {"m_in": m2[c], "s_in": r2.results[c]["s_out"],
         "dstrel": dstrel[c], "deg_col": deg_cols[c]} for c in range(N_CORES)], **tkw)
    LAST_RESULTS.append(r3)

    out = np.zeros((N_NODES, DIM_OUT), np.float32)
    for c in range(N_CORES):
        out[node_lists[c]] = r3.results[c]["out"][local_lists[c]]
    return out
